# revision 22
# baseline (speedup 1.0000x reference)
"""Trainium2 Bass kernel for Attention1D (visual-question attention).

Computation (per batch b):
    X2att = X @ W_ques + b_ques                      # [bs, 1024]
    Y2att = Y[b] @ W_vis                             # [512, 1024]
    att   = relu(Y2att + X2att[b])                   # [512, 1024]
    logits= att @ W_map (+ b_map, dropped: softmax shift-invariant)
    w     = softmax(logits)                          # [512]
    out[b]= w @ Y[b]                                 # [2048]

Strategy: data-parallel over batch across 8 cores (32 batches/core).
All matmuls in bf16 (fp32 PSUM accumulation). Host pre-shards and
pre-lays-out the inputs:
  - Y^T (v-major) bf16 for the main matmul rhs (contraction over v
    needs v on partitions); the same tile also feeds the final
    weighted sum on VectorE,
  - weights replicated, X^T for the X2att preamble.
On-chip per batch: 128 accumulating matmuls build att^T in PSUM,
ScalarE applies bias+relu into bf16 SBUF (X2att bias precomputed on PE
with b_ques folded in via a ones-row rank-1 matmul), 8 matmuls
contract att^T with W_map into logits[1, 512], softmax runs on
partition 0 (exp with hardware sum accumulator, DVE reciprocal; the
max-subtraction is dropped since logits are O(1) and softmax is
shift-invariant), the normalized weights are broadcast to all 128
partitions with a rank-1 matmul, and VectorE computes the weighted
sum over the resident Y^T tile (multiply + free-dim reduction), so Y
is read from HBM exactly once. A post-pass splits multi-wait
instructions into single-wait form (this walrus build allows one sync
wait per instruction).
"""

import numpy as np
import ml_dtypes

BS, N = 256, 512
QD, VD, AD = 2048, 2048, 1024
NCORES = 8
BPC = BS // NCORES  # batches per core
VC, QC, AC, NC_ = VD // 128, QD // 128, AD // 128, N // 128  # chunk counts

BF = ml_dtypes.bfloat16

_cache = {}


def _split_multiwait(nc, mybir):
    """walrus codegen in this toolchain supports a single sync-wait per
    instruction; hoist extra waits onto standalone same-engine
    EventSemaphore waits placed immediately before the instruction."""
    k = 0
    for f in nc.m.functions:
        for blk in f.blocks:
            il = blk.instructions
            new = []
            for inst in il:
                si = inst.sync_info
                waits = list(si.on_wait) if si and si.on_wait else []
                if len(waits) > 1:
                    for w in waits[:-1]:
                        k += 1
                        ev = mybir.InstEventSemaphore(
                            name=f"antsplitw_{k}",
                            engine=inst.engine,
                            ins=[],
                            outs=[],
                            sync_info=mybir.SyncInfo(on_wait=[w], on_update=[]),
                        )
                        nc.register_instruction(ev, overwrite=True)
                        new.append(ev)
                    si.on_wait = [waits[-1]]
                new.append(inst)
            il[:] = new


def _build_nc():
    import concourse.bass as bass
    import concourse.mybir as mybir
    from concourse import tile
    from contextlib import ExitStack

    f32, bf16 = mybir.dt.float32, mybir.dt.bfloat16
    AF = mybir.ActivationFunctionType

    nc = bass.Bass(target_bir_lowering=True)

    yt_d = nc.declare_dram_parameter("yt", [BPC, VC, 128, N], bf16, isOutput=False)
    wvis_d = nc.declare_dram_parameter("wvis", [VC, 128, AD], bf16, isOutput=False)
    wques_d = nc.declare_dram_parameter("wques", [QC, 128, AD], bf16, isOutput=False)
    wmap_d = nc.declare_dram_parameter("wmap", [128, AC], bf16, isOutput=False)
    bques_d = nc.declare_dram_parameter("bques", [128, AC], f32, isOutput=False)
    xt_d = nc.declare_dram_parameter("xt", [QC, 128, BPC], bf16, isOutput=False)
    ones_d = nc.declare_dram_parameter("ones", [1, 128], bf16, isOutput=False)
    bqrow_d = nc.declare_dram_parameter("bqrow", [1, AD], bf16, isOutput=False)
    out_d = nc.declare_dram_parameter("out", [BPC, VD], f32, isOutput=True)

    with tile.TileContext(nc) as tc, ExitStack() as ctx:
        consts = ctx.enter_context(tc.tile_pool(name="consts", bufs=1))
        yt_pool = ctx.enter_context(tc.tile_pool(name="yt", bufs=3))
        att_pool = ctx.enter_context(tc.tile_pool(name="att", bufs=2))
        sm_pool = ctx.enter_context(tc.tile_pool(name="sm", bufs=3))
        ob_pool = ctx.enter_context(tc.tile_pool(name="ob", bufs=2))
        psA = ctx.enter_context(tc.tile_pool(name="psA", bufs=3, space="PSUM"))
        psL = ctx.enter_context(tc.tile_pool(name="psL", bufs=2, space="PSUM"))
        psW = ctx.enter_context(tc.tile_pool(name="psW", bufs=2, space="PSUM"))

        # ---- load constants ----
        wvis_sb = consts.tile([128, VC * AD], bf16, tag="wvis")
        nc.sync.dma_start(
            wvis_sb.rearrange("p (v a) -> p v a", v=VC),
            wvis_d.rearrange("v p a -> p v a"),
        )
        wques_sb = consts.tile([128, QC * AD], bf16, tag="wques")
        nc.sync.dma_start(
            wques_sb.rearrange("p (q a) -> p q a", q=QC),
            wques_d.rearrange("q p a -> p q a"),
        )
        wmap_sb = consts.tile([128, AC], bf16, tag="wmap")
        nc.sync.dma_start(wmap_sb[:], wmap_d[:])
        ones_sb = consts.tile([1, 128], bf16, tag="ones")
        nc.sync.dma_start(ones_sb[:], ones_d[:])
        bqrow_sb = consts.tile([1, AD], bf16, tag="bqrow")
        nc.sync.dma_start(bqrow_sb[:], bqrow_d[:])
        xt_sb = consts.tile([128, QC * BPC], bf16, tag="xt")
        nc.sync.dma_start(
            xt_sb.rearrange("p (q b) -> p q b", q=QC),
            xt_d.rearrange("q p b -> p q b"),
        )

        # ---- preamble: X2att^T [a(8x128 chunks), BPC] fp32, bias folded in ----
        x2att_sb = consts.tile([128, AC * BPC], f32, tag="x2att")
        for a in range(AC):
            ps = psA.tile([128, BPC], f32, tag="main")
            for q in range(QC):
                nc.tensor.matmul(
                    ps[:],
                    wques_sb[:, q * AD + a * 128 : q * AD + (a + 1) * 128],
                    xt_sb[:, q * BPC : (q + 1) * BPC],
                    start=(q == 0),
                    stop=False,
                )
            nc.tensor.matmul(
                ps[:],
                bqrow_sb[0:1, a * 128 : (a + 1) * 128],
                ones_sb[0:1, 0:BPC],
                start=False,
                stop=True,
            )
            nc.vector.tensor_copy(
                x2att_sb[:, a * BPC : (a + 1) * BPC], ps[:]
            )
        # one-time ACT observer of DVE-produced x2att so later relu ACTs
        # carry only the PE wait (walrus ACT codegen allows 1 sync wait)
        x2obs = consts.tile([128, 1], f32, tag="x2obs")
        nc.scalar.copy(x2obs[:], x2att_sb[:, 0:1])

        # ---- main loop over batches ----
        for b in range(BPC):
            yt = yt_pool.tile([128, VC * N], bf16)
            nc.sync.dma_start(
                yt.rearrange("p (v n) -> p v n", v=VC),
                yt_d[b].rearrange("v p n -> p v n"),
            )
            att = att_pool.tile([128, AC * N], bf16)
            for a in range(AC):
                ps = psA.tile([128, N], f32, tag="main")
                for v in range(VC):
                    nc.tensor.matmul(
                        ps[:],
                        wvis_sb[:, v * AD + a * 128 : v * AD + (a + 1) * 128],
                        yt[:, v * N : (v + 1) * N],
                        start=(v == 0),
                        stop=(v == VC - 1),
                    )
                # att^T chunk = relu(psum + x2att[:, b]) -> bf16
                nc.scalar.activation(
                    att[:, a * N : (a + 1) * N],
                    ps[:],
                    AF.Relu,
                    bias=x2att_sb[:, a * BPC + b : a * BPC + b + 1],
                )

            # logits [1, N] = sum_a wmap[a_chunk]^T @ att^T[a_chunk]
            psl = psL.tile([1, N], f32)
            for a in range(AC):
                nc.tensor.matmul(
                    psl[:],
                    wmap_sb[:, a : a + 1],
                    att[:, a * N : (a + 1) * N],
                    start=(a == 0),
                    stop=(a == AC - 1),
                )

            # softmax on partition 0; logits are O(1) so exp needs no
            # max-subtraction (softmax is shift-invariant, fp32 exact enough)
            e_sb = sm_pool.tile([1, N], f32, tag="e")
            ssum = sm_pool.tile([1, 1], f32, tag="ssum")
            nc.scalar.activation(
                e_sb[:], psl[:], AF.Exp, accum_out=ssum[:]
            )
            rcp = sm_pool.tile([1, 1], f32, tag="rcp")
            nc.vector.reciprocal(rcp[:], ssum[:])
            rcp_a = sm_pool.tile([1, 1], f32, tag="rcp_a")
            nc.scalar.copy(rcp_a[:], rcp[:])
            e_w = sm_pool.tile([1, N], bf16, tag="e_w")
            nc.scalar.mul(e_w[:], e_sb[:], rcp_a[:, 0:1])

            # broadcast w = e/s to all 128 partitions via rank-1 matmul
            psw = psW.tile([128, N], f32)
            nc.tensor.matmul(psw[:], ones_sb[:], e_w[:], start=True, stop=True)
            wbc = sm_pool.tile([128, N], bf16, tag="wbc")
            nc.scalar.copy(wbc[:], psw[:])

            # weighted sum on DVE over the resident Y^T tile:
            # out^T[v_chunk*128+p] = sum_n Y^T[vp, n] * w[n]
            ob = ob_pool.tile([128, VC], f32)
            prod = sm_pool.tile([128, VC * N], bf16, tag="prod")
            nc.vector.tensor_tensor(
                prod[:].rearrange("p (c n) -> p c n", c=VC),
                yt[:].rearrange("p (c n) -> p c n", c=VC),
                wbc[:].rearrange("p (o n) -> p o n", o=1).broadcast_to(
                    [128, VC, N]
                ),
                op=mybir.AluOpType.mult,
            )
            nc.vector.reduce_sum(
                ob[:],
                prod[:].rearrange("p (c n) -> p c n", c=VC),
                axis=mybir.AxisListType.X,
            )
            nc.sync.dma_start(
                out_d[b].rearrange("(c p) -> p c", p=128), ob[:]
            )

    _split_multiwait(nc, mybir)
    return nc


def _prep_core_inputs(X, Y, W_vis, W_ques, b_ques, W_map):
    """Build per-core input maps (host-side shard + layout + bf16 cast)."""
    wvis_h = np.ascontiguousarray(W_vis.reshape(VC, 128, AD)).astype(BF)
    wques_h = np.ascontiguousarray(W_ques.reshape(QC, 128, AD)).astype(BF)
    wmap_h = np.ascontiguousarray(W_map.reshape(AC, 128).T).astype(BF)
    bques_h = np.ascontiguousarray(b_ques.reshape(AC, 128).T).astype(np.float32)
    bques_h_row = np.ascontiguousarray(b_ques.reshape(1, AD)).astype(BF)

    in_maps = []
    for c in range(NCORES):
        sl = slice(c * BPC, (c + 1) * BPC)
        Yc = Y[sl]  # [BPC, N, VD]
        yt = np.ascontiguousarray(Yc.transpose(0, 2, 1)).reshape(
            BPC, VC, 128, N
        ).astype(BF)
        xt = np.ascontiguousarray(X[sl].T).reshape(QC, 128, BPC).astype(BF)
        in_maps.append(
            {
                "yt": yt,
                "wvis": wvis_h,
                "wques": wques_h,
                "wmap": wmap_h,
                "bques": bques_h,
                "xt": xt,
                "ones": np.ones((1, 128), dtype=BF),
                "bqrow": bques_h_row,
            }
        )
    return in_maps


def _get_nc():
    if "nc" not in _cache:
        _cache["nc"] = _build_nc()
    return _cache["nc"]


def kernel(X, Y, W_vis, W_ques, b_ques, W_map, b_map, _trace=False):
    from concourse.bass_utils import run_bass_kernel_spmd

    X = np.asarray(X, dtype=np.float32)
    Y = np.asarray(Y, dtype=np.float32)
    in_maps = _prep_core_inputs(
        np.asarray(X), np.asarray(Y), np.asarray(W_vis),
        np.asarray(W_ques), np.asarray(b_ques), np.asarray(W_map)
    )
    nc = _get_nc()
    res = run_bass_kernel_spmd(
        nc, in_maps, core_ids=list(range(NCORES)), trace=_trace
    )
    if _trace:
        _cache["last_result"] = res
    out = np.concatenate([r["out"] for r in res.results], axis=0)
    # b_map shifts logits uniformly -> softmax-invariant; output unaffected.
    return out.astype(np.float32)
